# revision 23
# baseline (speedup 1.0000x reference)
"""Trainium2 Bass kernel for nn_BingramLanguageModel.

Model: x = tok_table[idx] + pos; single-head causal attention; lm head to
32000-way logits; cross-entropy loss. B=4, T=2048, C=256, V=32000.

Sharding: the 8192 = B*T rows are split across 8 cores; core c handles batch
b = c//2, rows [h*1024, h*1024+1024) with h = c%2.  To keep one SPMD program
for all cores, each core's token stream is ROTATED by its row offset (host
side), so the core's own 1024 query rows sit at local positions 0..1023
while all 2048 keys/values of the batch are available.  Attention scores are
computed transposed (S^T[s,t]) so the exp'd weights land directly in the
lhsT layout needed for wei @ v — no per-block transposes.  The softmax
denominator comes for free from a ones-column appended to v.  Causality:
blocks strictly below the diagonal need no mask; the diagonal block uses a
constant triangular mask; the rotated tail blocks ([1024,2048)) are all-keep
or all-drop per core, driven by a tiny per-core input column.  The LM head
streams Wl^T (host pre-transposed) with float32r matmuls; bias is added
during the PSUM->SBUF evacuation on DVE.  Loss is computed fully locally
per row (logsumexp via ScalarE exp+accumulate), reduced to one scalar per
core; the host sums 8 scalars.
"""

import numpy as np

import concourse.bass as bass
import concourse.bacc as bacc
import concourse.mybir as mybir
import concourse.tile as tile
from concourse.bass import IndirectOffsetOnAxis
from concourse.bass_utils import run_bass_kernel_spmd
from concourse.masks import make_identity

P = 128          # partitions
T = 2048         # sequence length (= batch rows)
C = 256          # embed dim
V = 32000        # vocab
TLOC = 1024      # rows per core
NT = T // P      # 16 key tiles
NTQ = TLOC // P  # 8 query tiles per core
VC = 1024        # vocab chunk width for the LM head
F32 = mybir.dt.float32
I32 = mybir.dt.int32
R32 = mybir.dt.float32r
BF16 = mybir.dt.bfloat16
AX = mybir.AxisListType
ALU = mybir.AluOpType
ACTF = mybir.ActivationFunctionType

VCHUNKS = [(j * VC, min(VC, V - j * VC)) for j in range((V + VC - 1) // VC)]
NEG = -1e30


def r32(ap):
    return ap.bitcast(R32)


def build_nc():
    nc = bacc.Bacc("TRN2", target_bir_lowering=False, debug=False, num_devices=8)

    tok = nc.dram_tensor("tok_table", [V, C], F32, kind="ExternalInput")
    pos = nc.dram_tensor("pos_rot", [T, C], F32, kind="ExternalInput")
    idxr = nc.dram_tensor("idx_rot", [T, 1], I32, kind="ExternalInput")
    tgt = nc.dram_tensor("targets_loc", [TLOC, 1], I32, kind="ExternalInput")
    wq_t = nc.dram_tensor("wq_t", [C, C], F32, kind="ExternalInput")
    wk_t = nc.dram_tensor("wk_t", [C, C], F32, kind="ExternalInput")
    wv_t = nc.dram_tensor("wv_t", [C, C], F32, kind="ExternalInput")
    wl_t = nc.dram_tensor("wl_t", [C, V], BF16, kind="ExternalInput")
    wl_rows = nc.dram_tensor("wl_rows", [V, C], F32, kind="ExternalInput")
    bl2 = nc.dram_tensor("bl2", [1, V], F32, kind="ExternalInput")
    bl_col = nc.dram_tensor("bl_col", [V, 1], F32, kind="ExternalInput")
    maskB = nc.dram_tensor("maskB", [P, 1], F32, kind="ExternalInput")

    logits = nc.dram_tensor("logits", [TLOC, V], BF16, kind="ExternalOutput")
    loss = nc.dram_tensor("loss", [1, 1], F32, kind="ExternalOutput")

    with tile.TileContext(nc) as tc:
        with tc.tile_pool(name="persist", bufs=1) as pp, \
             tc.tile_pool(name="wlstream", bufs=6) as wlp, \
             tc.tile_pool(name="blstream", bufs=2) as blp, \
             tc.tile_pool(name="blbcast", bufs=4) as blbp, \
             tc.tile_pool(name="logsb", bufs=4) as lgp, \
             tc.tile_pool(name="expscr", bufs=2) as exp_p, \
             tc.tile_pool(name="small", bufs=4) as smp:

            # ---------------- persistent SBUF ----------------
            xT = pp.tile([P, 2, T], R32)        # x^T  [cin-half, a, t]
            kT = pp.tile([P, 2, T], R32)        # k^T  [cout-half, a, s]
            qT = pp.tile([P, 2, TLOC], R32)     # q^T/sqrt(C) [cout-half, a, t]
            vsb = pp.tile([P, NT, C + 2], R32)  # v ++ ones col [s, tile, c]
            out_sb = pp.tile([P, NTQ, C], F32)  # attn out [t, tile, c]
            outT_t = [pp.tile([P, 2, P], BF16, name=f"outT{j}", tag=f"outT{j}")
                      for j in range(NTQ)]        # out^T per tile [c-half, a, t]
            wqs = pp.tile([P, 2, C], R32)       # Wq^T [ci-half, a, cout]
            wks = pp.tile([P, 2, C], R32)
            wvs = pp.tile([P, 2, C], R32)
            wq_st = pp.tile([P, 2, C], F32)
            wk_st = pp.tile([P, 2, C], F32)
            wv_st = pp.tile([P, 2, C], F32)
            ident = pp.tile([P, P], F32)
            ones_col = pp.tile([P, 1], F32)
            tri = pp.tile([P, P], F32)          # tri[s,t] = s>t ? NEG : 0
            mBc = pp.tile([P, 1], F32)          # 0 (keep) or NEG (drop) tail
            sums_lm = pp.tile([P, NTQ, (len(VCHUNKS) + 1) // 2], F32)
            loss_col = pp.tile([P, NTQ], F32)
            wlg = pp.tile([P, NTQ, C], F32)     # Wl rows of targets
            blg = pp.tile([P, NTQ], F32)        # bl[target]

            make_identity(nc, ident[:])
            nc.vector.memset(ones_col[:], 1.0)
            # tri[s, t]: keep (0) where t - s >= 0, else NEG
            nc.gpsimd.memset(tri[:], 0.0)
            nc.gpsimd.affine_select(
                out=tri[:], in_=tri[:], compare_op=ALU.is_ge, fill=NEG,
                base=0, pattern=[[1, P]], channel_multiplier=-1)
            for sb in range(NT):
                nc.vector.tensor_copy(vsb[:, sb, C:C + 1], ones_col[:])
                nc.vector.tensor_copy(vsb[:, sb, C + 1:C + 2], ones_col[:])

            nc.sync.dma_start(out=mBc[:], in_=maskB[:])
            nc.sync.dma_start(out=wq_st[:], in_=wq_t.rearrange("(a p) c -> p a c", p=P))
            nc.sync.dma_start(out=wk_st[:], in_=wk_t.rearrange("(a p) c -> p a c", p=P))
            nc.sync.dma_start(out=wv_st[:], in_=wv_t.rearrange("(a p) c -> p a c", p=P))
            nc.vector.tensor_copy(wqs[:], wq_st[:])
            nc.vector.tensor_copy(wks[:], wk_st[:])
            nc.vector.tensor_copy(wvs[:], wv_st[:])

            from contextlib import ExitStack
            es = ExitStack()
            pmm = es.enter_context(tc.tile_pool(name="mm512", bufs=2, space="PSUM"))
            pou = es.enter_context(tc.tile_pool(name="pout", bufs=2, space="PSUM"))
            emb = es.enter_context(tc.tile_pool(name="embed", bufs=2))
            weitp = es.enter_context(tc.tile_pool(name="weiT", bufs=12))
            plm = es.enter_context(tc.tile_pool(name="plm", bufs=2, space="PSUM"))

            # ------------- embeddings + x^T -------------
            for t in range(NT):
                it = emb.tile([P, 1], I32, tag="idx")
                nc.sync.dma_start(out=it[:], in_=idxr[t * P:(t + 1) * P, :])
                xt = emb.tile([P, C], F32, tag="x")
                nc.gpsimd.indirect_dma_start(
                    out=xt[:], out_offset=None, in_=tok[:],
                    in_offset=IndirectOffsetOnAxis(ap=it[:, :1], axis=0))
                pt = emb.tile([P, C], F32, tag="pos")
                nc.sync.dma_start(out=pt[:], in_=pos[t * P:(t + 1) * P, :])
                nc.vector.tensor_tensor(out=xt[:], in0=xt[:], in1=pt[:],
                                        op=ALU.add)
                for a in range(2):
                    tp = pmm.tile([P, P], F32, tag="mm")
                    nc.tensor.transpose(out=tp[:], in_=xt[:, a * P:(a + 1) * P],
                                        identity=ident[:])
                    nc.vector.tensor_copy(xT[:, a, t * P:(t + 1) * P], tp[:])

            # ------------- projections -------------
            for a in range(2):
                for s0 in range(0, T, 512):
                    pk = pmm.tile([P, 512], F32, tag="mm")
                    for c2 in range(2):
                        nc.tensor.matmul(
                            pk[:], lhsT=wks[:, c2, a * P:(a + 1) * P],
                            rhs=xT[:, c2, s0:s0 + 512],
                            start=(c2 == 0), stop=(c2 == 1))
                    nc.vector.tensor_copy(kT[:, a, s0:s0 + 512], pk[:])
                for s0 in range(0, TLOC, 512):
                    pq = pmm.tile([P, 512], F32, tag="mm")
                    for c2 in range(2):
                        nc.tensor.matmul(
                            pq[:], lhsT=wqs[:, c2, a * P:(a + 1) * P],
                            rhs=xT[:, c2, s0:s0 + 512],
                            start=(c2 == 0), stop=(c2 == 1))
                    nc.scalar.mul(qT[:, a, s0:s0 + 512], pq[:], C ** -0.5)
            for sb in range(NT):
                pv = pou.tile([P, C + 2], F32, tag="pou")
                for c2 in range(2):
                    nc.tensor.matmul(
                        pv[:, :C], lhsT=xT[:, c2, sb * P:(sb + 1) * P],
                        rhs=wvs[:, c2, :],
                        start=(c2 == 0), stop=(c2 == 1))
                nc.vector.tensor_copy(vsb[:, sb, :C], pv[:, :C])

            # ---- LM helper: process (pair jp, tile i) ----
            lm_pairs = {}

            def lm_prep(jp):
                pair = VCHUNKS[2 * jp: 2 * jp + 2]
                wlcs, blBs = [], []
                for (v0, vw) in pair:
                    wlc = wlp.tile([P, 2, VC], BF16, tag="wlc", name=f"wlc{v0}")
                    nc.sync.dma_start(
                        out=wlc[:, :, :vw],
                        in_=wl_t.rearrange("(a p) v -> p a v", p=P)[:, :, v0:v0 + vw])
                    wlcs.append(wlc)
                    blc = blp.tile([1, VC], F32, tag="blc", name=f"blc{v0}")
                    nc.sync.dma_start(out=blc[:, :vw], in_=bl2[:, v0:v0 + vw])
                    blB = blbp.tile([P, VC], F32, tag="blB", name=f"blB{v0}")
                    nc.gpsimd.partition_broadcast(blB[:, :vw], blc[:1, :vw])
                    blBs.append(blB)
                lm_pairs[jp] = (pair, wlcs, blBs)

            def lm_tile(jp, i):
                pair, wlcs, blBs = lm_pairs[jp]
                lsb = lgp.tile([P, 2 * VC], BF16, tag="lsb",
                               name=f"lsb{jp}_{i}")
                tw = 0
                for ci, (v0, vw) in enumerate(pair):
                    pl = plm.tile([P, VC], F32, tag="lm",
                                  name=f"pl{jp}_{i}_{ci}")
                    for a in range(2):
                        for n0 in range(0, vw, 512):
                            nw = min(512, vw - n0)
                            nc.tensor.matmul(
                                pl[:, n0:n0 + nw], lhsT=outT_t[i][:, a, :],
                                rhs=wlcs[ci][:, a, n0:n0 + nw],
                                start=(a == 0), stop=(a == 1))
                    nc.vector.tensor_tensor(
                        out=lsb[:, ci * VC:ci * VC + vw], in0=pl[:, :vw],
                        in1=blBs[ci][:, :vw], op=ALU.add)
                    nc.sync.dma_start(
                        out=logits[i * P:(i + 1) * P, v0:v0 + vw],
                        in_=lsb[:, ci * VC:ci * VC + vw])
                    tw = ci * VC + vw
                esc = exp_p.tile([P, 2 * VC], F32, tag="esc",
                                 name=f"esc{jp}_{i}")
                nc.scalar.activation(
                    esc[:, :tw], lsb[:, :tw], ACTF.Exp,
                    accum_out=sums_lm[:, i, jp:jp + 1])

            lm_prep(0)
            lm_prep(1)

            # ------------- attention (S^T, q-tile pairs) -------------
            for jj in range(0, NTQ, 2):
                j0, j1 = jj, jj + 1
                common = list(range(0, j0 + 1)) + list(range(NTQ, NT))
                wts = {}
                for sb in common:
                    st = pmm.tile([P, 2 * P], F32, tag="mm",
                                  name=f"st{jj}_{sb}")
                    for a in range(2):
                        nc.tensor.matmul(
                            st[:], lhsT=kT[:, a, sb * P:(sb + 1) * P],
                            rhs=qT[:, a, j0 * P:(j0 + 2) * P],
                            start=(a == 0), stop=(a == 1))
                    if sb == j0:
                        nc.vector.tensor_tensor(out=st[:, :P], in0=st[:, :P],
                                                in1=tri[:], op=ALU.add)
                    elif sb >= NTQ:
                        nc.vector.tensor_scalar_add(st[:], st[:], mBc[:, :1])
                    wt = weitp.tile([P, 2 * P], R32, tag="weiT",
                                    name=f"wt{jj}_{sb}")
                    nc.scalar.activation(wt[:], st[:], ACTF.Exp)
                    wts[sb] = wt
                # extra diagonal block for tile j1
                st2 = pmm.tile([P, 2 * P], F32, tag="mm", name=f"st2_{jj}")
                for a in range(2):
                    nc.tensor.matmul(
                        st2[:, :P], lhsT=kT[:, a, j1 * P:(j1 + 1) * P],
                        rhs=qT[:, a, j1 * P:(j1 + 1) * P],
                        start=(a == 0), stop=(a == 1))
                nc.vector.tensor_tensor(out=st2[:, :P], in0=st2[:, :P],
                                        in1=tri[:], op=ALU.add)
                wt2 = weitp.tile([P, 2 * P], R32, tag="weiT",
                                 name=f"wt2_{jj}")
                nc.scalar.activation(wt2[:, :P], st2[:, :P], ACTF.Exp)

                for half, j in ((0, j0), (1, j1)):
                    po = pou.tile([P, C + 2], F32, tag="pou",
                                  name=f"po{j}")
                    blocks = list(range(0, j + 1)) + list(range(NTQ, NT))
                    for bi, sb in enumerate(blocks):
                        if sb == j1 and half == 1:
                            lh = wt2[:, :P]
                        else:
                            lh = wts[sb][:, half * P:(half + 1) * P]
                        nc.tensor.matmul(
                            po[:], lhsT=lh, rhs=vsb[:, sb, :],
                            start=(bi == 0), stop=(bi == len(blocks) - 1))
                    rinv = smp.tile([P, 1], F32, tag="rinv")
                    nc.vector.reciprocal(rinv[:], po[:, C:C + 1])
                    nc.vector.tensor_scalar_mul(out_sb[:, j, :], po[:, :C],
                                                rinv[:, :1])
                    for a in range(2):
                        tp2 = pmm.tile([P, 2 * P], F32, tag="mm",
                                       name=f"tp2_{j}_{a}")
                        nc.tensor.transpose(
                            out=tp2[:, :P], in_=out_sb[:, j, a * P:(a + 1) * P],
                            identity=ident[:])
                        nc.scalar.copy(outT_t[j][:, a, :], tp2[:, :P])
                    lm_tile(0, j)
                    lm_tile(1, j)

            # target-row gathers (for the loss)
            for i in range(NTQ):
                tg = emb.tile([P, 1], I32, tag="idx")
                nc.sync.dma_start(out=tg[:], in_=tgt[i * P:(i + 1) * P, :])
                nc.gpsimd.indirect_dma_start(
                    out=wlg[:, i, :], out_offset=None, in_=wl_rows[:],
                    in_offset=IndirectOffsetOnAxis(ap=tg[:, :1], axis=0))
                nc.gpsimd.indirect_dma_start(
                    out=blg[:, i:i + 1], out_offset=None,
                    in_=bl_col[:],
                    in_offset=IndirectOffsetOnAxis(ap=tg[:, :1], axis=0))

            # ---------------- LM head (remaining chunk pairs) ----------------
            NPAIR = (len(VCHUNKS) + 1) // 2
            for jp in range(2, NPAIR):
                lm_prep(jp)
                for i in range(NTQ):
                    lm_tile(jp, i)

            # ---------------- loss ----------------
            for i in range(NTQ):
                stot = smp.tile([P, 1], F32, tag="lstot")
                nc.vector.tensor_reduce(
                    out=stot[:], in_=sums_lm[:, i, :],
                    axis=AX.X, op=ALU.add)
                lse = smp.tile([P, 1], F32, tag="lse")
                nc.scalar.activation(lse[:], stot[:], ACTF.Ln)
                dot_scr = smp.tile([P, C], F32, tag="dscr")
                tdot = smp.tile([P, 1], F32, tag="tdot")
                nc.vector.tensor_tensor(
                    out=dot_scr[:], in0=out_sb[:, i, :], in1=wlg[:, i, :],
                    op=ALU.mult)
                nc.vector.tensor_reduce(
                    out=tdot[:], in_=dot_scr[:], axis=AX.X, op=ALU.add)
                nc.vector.tensor_tensor(out=tdot[:], in0=tdot[:],
                                        in1=blg[:, i:i + 1], op=ALU.add)
                nc.vector.tensor_tensor(out=loss_col[:, i:i + 1], in0=lse[:],
                                        in1=tdot[:], op=ALU.subtract)

            es.close()
            with tc.tile_pool(name="plast", bufs=1, space="PSUM") as pla:
                lvec = smp.tile([P, 1], F32, tag="lvec")
                nc.vector.tensor_reduce(out=lvec[:], in_=loss_col[:],
                                        axis=AX.X, op=ALU.add)
                pls = pla.tile([1, 1], F32)
                nc.tensor.matmul(pls[:], lhsT=lvec[:], rhs=ones_col[:],
                                 start=True, stop=True)
                lss = smp.tile([1, 1], F32, tag="lss")
                nc.vector.tensor_copy(lss[:], pls[:])
                nc.sync.dma_start(out=loss[:], in_=lss[:])

    nc.compile()
    return nc


_NC_CACHE = None


def _get_nc():
    global _NC_CACHE
    if _NC_CACHE is None:
        _NC_CACHE = build_nc()
    return _NC_CACHE


def make_in_maps(idx, targets, tok_table, pos_table, Wq, Wk, Wv, Wl, bl):
    idx = np.asarray(idx)
    targets = np.asarray(targets)
    tok_table = np.ascontiguousarray(np.asarray(tok_table), dtype=np.float32)
    pos_table = np.asarray(pos_table, dtype=np.float32)
    Wq = np.asarray(Wq, dtype=np.float32)
    Wk = np.asarray(Wk, dtype=np.float32)
    Wv = np.asarray(Wv, dtype=np.float32)
    Wl = np.ascontiguousarray(np.asarray(Wl), dtype=np.float32)
    bl = np.asarray(bl, dtype=np.float32)

    wq_t = np.ascontiguousarray(Wq.T)
    wk_t = np.ascontiguousarray(Wk.T)
    wv_t = np.ascontiguousarray(Wv.T)
    import ml_dtypes
    wl_t = np.ascontiguousarray(Wl.T.astype(ml_dtypes.bfloat16))
    bl2 = np.ascontiguousarray(bl.reshape(1, V))
    bl_col_arr = np.ascontiguousarray(bl.reshape(V, 1))

    in_maps = []
    for c in range(8):
        b, h = c // 2, c % 2
        tb = h * TLOC
        idx_rot = np.roll(idx[b].astype(np.int32), -tb)
        pos_rot = np.ascontiguousarray(np.roll(pos_table[:T], -tb, axis=0))
        in_maps.append({
            "tok_table": tok_table,
            "pos_rot": pos_rot,
            "idx_rot": np.ascontiguousarray(idx_rot.reshape(T, 1)),
            "targets_loc": np.ascontiguousarray(
                targets[b, tb:tb + TLOC].astype(np.int32).reshape(TLOC, 1)),
            "wq_t": wq_t, "wk_t": wk_t, "wv_t": wv_t,
            "wl_t": wl_t, "wl_rows": Wl, "bl2": bl2,
            "bl_col": bl_col_arr,
            "maskB": np.full((P, 1), 0.0 if h == 1 else NEG, np.float32),
        })
    return in_maps


def assemble(results):
    logits_flat = np.empty((4 * T, V), np.float32)
    loss_sum = 0.0
    for c, r in enumerate(results):
        b, h = c // 2, c % 2
        tb = h * TLOC
        logits_flat[b * T + tb: b * T + tb + TLOC] = \
            np.asarray(r["logits"]).astype(np.float32)
        loss_sum += float(r["loss"][0, 0])
    return logits_flat, np.float32(loss_sum / (4 * T))


def kernel(idx, targets, tok_table, pos_table, Wq, Wk, Wv, Wl, bl):
    nc = _get_nc()
    in_maps = make_in_maps(idx, targets, tok_table, pos_table, Wq, Wk, Wv, Wl, bl)
    res = run_bass_kernel_spmd(nc, in_maps, list(range(8)))
    return assemble(res.results)
